# revision 3
# baseline (speedup 1.0000x reference)
"""BankedLinear (MoE-style banked linear) Trainium2 kernel.

Reference computation (per token t, with k=2 selected banks):
    out[t] = sum_k prob[t,k] * (x[t] @ W[sel[t,k]] + bias[sel[t,k]])

Strategy (expert-parallel over 8 NeuronCores):
  - Core c owns banks [8c, 8c+8).  Its weight slab (8 x 512 x 512 fp32 = 8 MB)
    is the dominant, unavoidable HBM traffic; each bank is read exactly once
    system-wide.
  - Host routes token-bank pairs to cores by selected bank, pre-scales each
    gathered token row by its probability, and lays rows out transposed
    ([in_feature, slot]) so they can feed the PE as the stationary operand.
  - Device: per bank, 4 accumulating matmuls [128,64].T @ [128,512] -> PSUM,
    copy to SBUF, DMA out.  Bias is folded in on the host (cheap: one gather
    + multiply-add over 1024 pairs).
  - Host scatter-adds the per-pair results into the output.

Fixed shapes: B=2, T=256, K=2, IN=OUT=512, NB=64 banks, 8 cores.
Capacity padding: 64 slots per bank (binomial mean 16, sd ~4 -> overflow is
~12-sigma; any overflow pairs are handled exactly on the host as a fallback).
"""

import numpy as np
from contextlib import ExitStack

B, T, KSEL = 2, 256, 2
IN, OUT, NB = 512, 512, 64
NCORES = 8
BPC = NB // NCORES          # banks per core = 8
CAP = 64                    # padded token slots per bank
SLOTS = BPC * CAP           # 512 dispatch rows per core
PCHUNK = 128                # contraction chunk (SBUF partition dim)
KC = IN // PCHUNK           # 4 contraction chunks

_cache = {}


def _build_nc():
    """Build the Bass/Tile program (one SPMD NeuronCore program)."""
    import concourse.tile as tile
    import concourse.mybir as mybir
    from concourse import bacc

    f32 = mybir.dt.float32
    nc = bacc.Bacc("TRN2", target_bir_lowering=False, debug=False,
                   num_devices=NCORES)
    xt = nc.dram_tensor("xt", [IN, SLOTS], f32, kind="ExternalInput").ap()
    w = nc.dram_tensor("w", [BPC, IN, OUT], f32, kind="ExternalInput").ap()
    y = nc.dram_tensor("y", [SLOTS, OUT], f32, kind="ExternalOutput").ap()

    with tile.TileContext(nc) as tc:
        with ExitStack() as ctx:
            xpool = ctx.enter_context(tc.tile_pool(name="xp", bufs=KC))
            wpool = ctx.enter_context(tc.tile_pool(name="wp", bufs=BPC * KC))
            ypool = ctx.enter_context(tc.tile_pool(name="yp", bufs=4))
            pspool = ctx.enter_context(
                tc.tile_pool(name="ps", bufs=4, space="PSUM"))

            # Dispatched tokens, transposed: xt[in_feature, slot].
            xts = []
            for kc in range(KC):
                t = xpool.tile([PCHUNK, SLOTS], f32, tag="xt")
                nc.sync.dma_start(t[:], xt[kc * PCHUNK:(kc + 1) * PCHUNK, :])
                xts.append(t)

            for j in range(BPC):
                wts = []
                for kc in range(KC):
                    wt = wpool.tile([PCHUNK, OUT], f32, tag="w")
                    nc.sync.dma_start(
                        wt[:], w[j, kc * PCHUNK:(kc + 1) * PCHUNK, :])
                    wts.append(wt)
                ps = pspool.tile([CAP, OUT], f32, tag="ps")
                for kc in range(KC):
                    nc.tensor.matmul(
                        ps[:],
                        xts[kc][:, j * CAP:(j + 1) * CAP],   # lhsT [128, 64]
                        wts[kc][:],                          # rhs  [128, 512]
                        start=(kc == 0), stop=(kc == KC - 1))
                ysb = ypool.tile([CAP, OUT], f32, tag="y")
                nc.vector.tensor_copy(ysb[:], ps[:])
                nc.sync.dma_start(y[j * CAP:(j + 1) * CAP, :], ysb[:])
    nc.compile()
    return nc


def _get_nc():
    if "nc" not in _cache:
        _cache["nc"] = _build_nc()
    return _cache["nc"]


def _route(X, sel, prob):
    """Group token-bank pairs by bank, build per-core dispatch arrays.

    Returns (in_xt [NCORES,IN,SLOTS], slot_tok [NCORES,SLOTS] int64 (-1=pad),
    overflow list of (token, bank, prob))."""
    NT = X.shape[0]
    pair_tok = np.repeat(np.arange(NT, dtype=np.int64), KSEL)
    pair_bank = sel.reshape(-1)
    pair_p = prob.reshape(-1)

    order = np.argsort(pair_bank, kind="stable")
    counts = np.bincount(pair_bank, minlength=NB)
    starts = np.concatenate(([0], np.cumsum(counts)))

    slot_tok = np.full((NCORES, SLOTS), -1, dtype=np.int64)
    slot_p = np.zeros((NCORES, SLOTS), dtype=np.float32)
    overflow = []
    for b in range(NB):
        c, j = divmod(b, BPC)
        s0, s1 = starts[b], starts[b + 1]
        take = min(s1 - s0, CAP)
        idx = order[s0:s0 + take]
        slot_tok[c, j * CAP: j * CAP + take] = pair_tok[idx]
        slot_p[c, j * CAP: j * CAP + take] = pair_p[idx]
        for i in order[s0 + take:s1]:
            overflow.append((int(pair_tok[i]), b, float(pair_p[i])))

    in_xt = np.empty((NCORES, IN, SLOTS), dtype=np.float32)
    for c in range(NCORES):
        tok = slot_tok[c]
        rows = X[np.where(tok >= 0, tok, 0)] * slot_p[c][:, None]
        in_xt[c] = np.ascontiguousarray(rows.T)
    return in_xt, slot_tok, overflow


def _combine(ys, slot_tok, X, sel, prob, weights, bias, overflow):
    NT = X.shape[0]
    out = np.zeros((NT, OUT), dtype=np.float32)
    for c in range(NCORES):
        tok = slot_tok[c]
        valid = tok >= 0
        np.add.at(out, tok[valid], ys[c][valid])
    # bias term for every pair (device computes x @ W only)
    for k in range(KSEL):
        out += prob[:, k, None] * bias[sel[:, k]]
    # exact host fallback for capacity-overflow pairs (expected: none)
    for t, b, p in overflow:
        out[t] += p * (X[t] @ weights[b])
    return out


def _run_device(in_maps, trace=False, **kwargs):
    from concourse.bass_utils import run_bass_kernel_spmd
    return run_bass_kernel_spmd(_get_nc(), in_maps,
                                core_ids=list(range(NCORES)),
                                trace=trace, **kwargs)


def kernel(_trace=False, _bass_results=None, **inputs):
    tensor = np.asarray(inputs["tensor"], dtype=np.float32)
    sel = np.asarray(inputs["bank_selections"]).astype(np.int64)
    prob = np.asarray(inputs["bank_probabilities"], dtype=np.float32)
    weights = np.asarray(inputs["weights"], dtype=np.float32)
    bias = np.asarray(inputs["bias"], dtype=np.float32)

    NT = tensor.shape[0] * tensor.shape[1]
    X = tensor.reshape(NT, IN)
    sel2 = sel.reshape(NT, KSEL)
    prob2 = prob.reshape(NT, KSEL)

    in_xt, slot_tok, overflow = _route(X, sel2, prob2)
    in_maps = [
        {"xt": in_xt[c],
         "w": np.ascontiguousarray(weights[c * BPC:(c + 1) * BPC])}
        for c in range(NCORES)
    ]
    res = _run_device(in_maps, trace=_trace)
    if _bass_results is not None:
        _bass_results.append(res)
    ys = [res.results[c]["y"] for c in range(NCORES)]

    out = _combine(ys, slot_tok, X, sel2, prob2, weights, bias, overflow)
    return out.reshape(tensor.shape[0], tensor.shape[1], OUT)


# revision 5
# speedup vs baseline: 1.1661x; 1.1661x over previous
"""BankedLinear (MoE-style banked linear) Trainium2 kernel.

Reference computation (per token t, with k=2 selected banks):
    out[t] = sum_k prob[t,k] * (x[t] @ W[sel[t,k]] + bias[sel[t,k]])

Strategy (expert-parallel over 8 NeuronCores):
  - Core c owns banks [8c, 8c+8).  Its weight slab (8 x 512 x 512 = 8 MB of
    fp32 information) is the dominant, unavoidable HBM traffic; each bank is
    read exactly once system-wide.
  - Host routes token-bank pairs to cores by selected bank, pre-scales each
    gathered token row by its probability, transposes to [in_feature, slot],
    and pads to CAP=32 slots per bank.
  - Precision/speed: fp32 matmul runs at 1/4 rate on the PE and bf16 at full
    rate, so both x and W are split hi/lo into two bf16 halves on the host
    (same total bytes as fp32) and each bank's product is computed as
    xh@wh + xh@wl + xl@wh accumulated in fp32 PSUM (~1e-6 rel error).
  - All arrays are pre-swizzled on the host into SBUF layout so every DMA is
    a single large contiguous 2D transfer.
  - Bias is folded in on the host (one gather + multiply-add over 1024
    pairs); host scatter-adds the per-pair device results into the output.

Fixed shapes: B=2, T=256, K=2, IN=OUT=512, NB=64 banks, 8 cores.
Capacity: 32 slots/bank (binomial mean 16, sd ~4; overflow pairs — none for
realistic routing — are handled exactly on the host as a fallback).
"""

import numpy as np
from contextlib import ExitStack

B, T, KSEL = 2, 256, 2
IN, OUT, NB = 512, 512, 64
NCORES = 8
BPC = NB // NCORES          # banks per core = 8
CAP = 32                    # padded token slots per bank
SLOTS = BPC * CAP           # 256 dispatch rows per core
PCHUNK = 128                # contraction chunk (SBUF partition dim)
KC = IN // PCHUNK           # 4 contraction chunks
GROUPS = SLOTS // 128       # output row groups of 128

_cache = {}


def _build_nc():
    """Build the Bass/Tile program (one SPMD NeuronCore program)."""
    import concourse.tile as tile
    import concourse.mybir as mybir
    from concourse import bacc

    f32 = mybir.dt.float32
    bf16 = mybir.dt.bfloat16
    nc = bacc.Bacc("TRN2", target_bir_lowering=False, debug=False,
                   num_devices=NCORES)
    # host-pre-swizzled SBUF layouts: partition dim first, contiguous free dim
    xth = nc.dram_tensor("xth", [PCHUNK, KC * SLOTS], bf16,
                         kind="ExternalInput").ap()
    xtl = nc.dram_tensor("xtl", [PCHUNK, KC * SLOTS], bf16,
                         kind="ExternalInput").ap()
    wh = nc.dram_tensor("wh", [BPC, PCHUNK, KC * OUT], bf16,
                        kind="ExternalInput").ap()
    wl = nc.dram_tensor("wl", [BPC, PCHUNK, KC * OUT], bf16,
                        kind="ExternalInput").ap()
    y = nc.dram_tensor("y", [SLOTS, OUT], f32, kind="ExternalOutput").ap()

    with tile.TileContext(nc) as tc:
        with ExitStack() as ctx:
            xpool = ctx.enter_context(tc.tile_pool(name="xp", bufs=2))
            wpool = ctx.enter_context(tc.tile_pool(name="wp", bufs=2 * BPC))
            ypool = ctx.enter_context(tc.tile_pool(name="yp", bufs=GROUPS))
            pspool = ctx.enter_context(
                tc.tile_pool(name="ps", bufs=6, space="PSUM"))

            xh_sb = xpool.tile([PCHUNK, KC * SLOTS], bf16, tag="xh")
            nc.sync.dma_start(xh_sb[:], xth[:])
            xl_sb = xpool.tile([PCHUNK, KC * SLOTS], bf16, tag="xl")
            nc.sync.dma_start(xl_sb[:], xtl[:])

            ysbs = []
            for g in range(GROUPS):
                ysb_g = ypool.tile([128, OUT], f32, tag="y")
                ysbs.append(ysb_g)

            for j in range(BPC):
                wh_sb = wpool.tile([PCHUNK, KC * OUT], bf16, tag="w")
                nc.sync.dma_start(wh_sb[:], wh[j])
                wl_sb = wpool.tile([PCHUNK, KC * OUT], bf16, tag="w")
                nc.sync.dma_start(wl_sb[:], wl[j])

                ps = pspool.tile([CAP, OUT], f32, tag="ps")
                nmm = 3 * KC
                i = 0
                for kc in range(KC):
                    xs = slice(kc * SLOTS + j * CAP, kc * SLOTS + (j + 1) * CAP)
                    ws = slice(kc * OUT, (kc + 1) * OUT)
                    for a_sb, b_sb in ((xh_sb, wh_sb), (xh_sb, wl_sb),
                                       (xl_sb, wh_sb)):
                        nc.tensor.matmul(ps[:], a_sb[:, xs], b_sb[:, ws],
                                         start=(i == 0), stop=(i == nmm - 1))
                        i += 1
                g, q = divmod(j, 128 // CAP)
                nc.vector.tensor_copy(
                    ysbs[g][q * CAP:(q + 1) * CAP, :], ps[:])

            for g in range(GROUPS):
                nc.sync.dma_start(y[g * 128:(g + 1) * 128, :], ysbs[g][:])
    nc.compile()
    return nc


def _get_nc():
    if "nc" not in _cache:
        _cache["nc"] = _build_nc()
    return _cache["nc"]


def _split_hilo(a32):
    """fp32 array -> (hi, lo) bf16 halves with a32 ~= hi + lo."""
    import ml_dtypes
    bf = ml_dtypes.bfloat16
    hi = a32.astype(bf)
    lo = (a32 - hi.astype(np.float32)).astype(bf)
    return hi, lo


def _swizzle_x(xt):
    """[IN, SLOTS] -> [128, KC*SLOTS] with free index (kc, slot)."""
    return np.ascontiguousarray(
        xt.reshape(KC, PCHUNK, SLOTS).transpose(1, 0, 2).reshape(
            PCHUNK, KC * SLOTS))


def _swizzle_w(w):
    """[BPC, IN, OUT] -> [BPC, 128, KC*OUT] with free index (kc, out)."""
    return np.ascontiguousarray(
        w.reshape(BPC, KC, PCHUNK, OUT).transpose(0, 2, 1, 3).reshape(
            BPC, PCHUNK, KC * OUT))


def _route(X, sel, prob):
    """Group token-bank pairs by bank, build per-core dispatch arrays.

    Returns (in_maps, slot_tok [NCORES,SLOTS] int64 (-1=pad), overflow list
    of (token, bank, prob))."""
    NT = X.shape[0]
    pair_tok = np.repeat(np.arange(NT, dtype=np.int64), KSEL)
    pair_bank = sel.reshape(-1)
    pair_p = prob.reshape(-1)

    order = np.argsort(pair_bank, kind="stable")
    counts = np.bincount(pair_bank, minlength=NB)
    starts = np.concatenate(([0], np.cumsum(counts)))

    slot_tok = np.full((NCORES, SLOTS), -1, dtype=np.int64)
    slot_p = np.zeros((NCORES, SLOTS), dtype=np.float32)
    overflow = []
    for b in range(NB):
        c, j = divmod(b, BPC)
        s0, s1 = starts[b], starts[b + 1]
        take = min(s1 - s0, CAP)
        idx = order[s0:s0 + take]
        slot_tok[c, j * CAP: j * CAP + take] = pair_tok[idx]
        slot_p[c, j * CAP: j * CAP + take] = pair_p[idx]
        for i in order[s0 + take:s1]:
            overflow.append((int(pair_tok[i]), b, float(pair_p[i])))
    return slot_tok, slot_p, overflow


def _combine(ys, slot_tok, X, sel, prob, weights, bias, overflow):
    NT = X.shape[0]
    out = np.zeros((NT, OUT), dtype=np.float32)
    for c in range(NCORES):
        tok = slot_tok[c]
        valid = tok >= 0
        np.add.at(out, tok[valid], ys[c][valid])
    # bias term for every pair (device computes x @ W only)
    for k in range(KSEL):
        out += prob[:, k, None] * bias[sel[:, k]]
    # exact host fallback for capacity-overflow pairs (expected: none)
    for t, b, p in overflow:
        out[t] += p * (X[t] @ weights[b])
    return out


def _run_device(in_maps, trace=False, **kwargs):
    from concourse.bass_utils import run_bass_kernel_spmd
    return run_bass_kernel_spmd(_get_nc(), in_maps,
                                core_ids=list(range(NCORES)),
                                trace=trace, **kwargs)


def kernel(_trace=False, _bass_results=None, **inputs):
    tensor = np.asarray(inputs["tensor"], dtype=np.float32)
    sel = np.asarray(inputs["bank_selections"]).astype(np.int64)
    prob = np.asarray(inputs["bank_probabilities"], dtype=np.float32)
    weights = np.asarray(inputs["weights"], dtype=np.float32)
    bias = np.asarray(inputs["bias"], dtype=np.float32)

    NT = tensor.shape[0] * tensor.shape[1]
    X = tensor.reshape(NT, IN)
    sel2 = sel.reshape(NT, KSEL)
    prob2 = prob.reshape(NT, KSEL)

    slot_tok, slot_p, overflow = _route(X, sel2, prob2)

    in_maps = []
    for c in range(NCORES):
        tok = slot_tok[c]
        rows = X[np.where(tok >= 0, tok, 0)] * slot_p[c][:, None]
        xt = np.ascontiguousarray(rows.T)              # [IN, SLOTS] fp32
        xh, xl = _split_hilo(xt)
        w32 = weights[c * BPC:(c + 1) * BPC]           # (8, 512, 512) fp32
        wwh, wwl = _split_hilo(w32)
        in_maps.append({
            "xth": _swizzle_x(xh), "xtl": _swizzle_x(xl),
            "wh": _swizzle_w(wwh), "wl": _swizzle_w(wwl),
        })

    res = _run_device(in_maps, trace=_trace)
    if _bass_results is not None:
        _bass_results.append(res)
    ys = [res.results[c]["y"] for c in range(NCORES)]

    out = _combine(ys, slot_tok, X, sel2, prob2, weights, bias, overflow)
    return out.reshape(tensor.shape[0], tensor.shape[1], OUT)


# revision 10
# speedup vs baseline: 1.1689x; 1.0024x over previous
"""BankedLinear (MoE-style banked linear) Trainium2 kernel.

Reference computation (per token t, with k=2 selected banks):
    out[t] = sum_k prob[t,k] * (x[t] @ W[sel[t,k]] + bias[sel[t,k]])

Strategy (expert-parallel over 8 NeuronCores):
  - Core c owns banks [8c, 8c+8).  Its weight slab (8 x 512 x 512 = 8 MB of
    fp32 information) is the dominant, unavoidable HBM traffic; each bank is
    read exactly once system-wide.
  - Host routes token-bank pairs to cores by selected bank, pre-scales each
    gathered token row by its probability, transposes to [in_feature, slot],
    and pads to CAP=32 slots per bank.
  - Precision/speed: fp32 matmul runs at 1/4 rate on the PE and bf16 at full
    rate, so both x and W are split hi/lo into two bf16 halves on the host
    (same total bytes as fp32) and each bank's product is computed as
    xh@wh + xh@wl + xl@wh accumulated in fp32 PSUM (~1e-6 rel error).
  - All arrays are pre-swizzled on the host into SBUF layout so every DMA is
    a single large contiguous 2D transfer.
  - Bias is folded in on the host (one gather + multiply-add over 1024
    pairs); host scatter-adds the per-pair device results into the output.

Fixed shapes: B=2, T=256, K=2, IN=OUT=512, NB=64 banks, 8 cores.
Capacity: 32 slots/bank (binomial mean 16, sd ~4; overflow pairs — none for
realistic routing — are handled exactly on the host as a fallback).
"""

import numpy as np
from contextlib import ExitStack

B, T, KSEL = 2, 256, 2
IN, OUT, NB = 512, 512, 64
NCORES = 8
BPC = NB // NCORES          # banks per core = 8
CAP = 32                    # padded token slots per bank
SLOTS = BPC * CAP           # 256 dispatch rows per core
PCHUNK = 128                # contraction chunk (SBUF partition dim)
KC = IN // PCHUNK           # 4 contraction chunks
GROUPS = SLOTS // 128       # output row groups of 128

_cache = {}


def _build_nc():
    """Build the Bass/Tile program (one SPMD NeuronCore program)."""
    import concourse.tile as tile
    import concourse.mybir as mybir
    from concourse import bacc

    f32 = mybir.dt.float32
    bf16 = mybir.dt.bfloat16
    nc = bacc.Bacc("TRN2", target_bir_lowering=False, debug=False,
                   num_devices=NCORES)
    # host-pre-swizzled SBUF layouts: partition dim first, contiguous free dim
    xth = nc.dram_tensor("xth", [PCHUNK, KC * SLOTS], bf16,
                         kind="ExternalInput").ap()
    xtl = nc.dram_tensor("xtl", [PCHUNK, KC * SLOTS], bf16,
                         kind="ExternalInput").ap()
    wh = nc.dram_tensor("wh", [BPC, PCHUNK, KC * OUT], bf16,
                        kind="ExternalInput").ap()
    wl = nc.dram_tensor("wl", [BPC, PCHUNK, KC * OUT], bf16,
                        kind="ExternalInput").ap()
    y = nc.dram_tensor("y", [SLOTS, OUT], f32, kind="ExternalOutput").ap()

    with tile.TileContext(nc) as tc:
        with ExitStack() as ctx:
            xpool = ctx.enter_context(tc.tile_pool(name="xp", bufs=2))
            wpool = ctx.enter_context(tc.tile_pool(name="wp", bufs=2 * BPC))
            ypool = ctx.enter_context(tc.tile_pool(name="yp", bufs=GROUPS))
            warmpool = ctx.enter_context(tc.tile_pool(name="wu", bufs=1))
            pspool = ctx.enter_context(
                tc.tile_pool(name="ps", bufs=6, space="PSUM"))
            pswarm = ctx.enter_context(
                tc.tile_pool(name="psw", bufs=1, space="PSUM"))

            # PE warm-up: ~10 dummy matmuls on memset data so the HAM clock
            # gate reaches 8/8 before the real matmuls start (PE is idle
            # during the weight DMA window anyway).
            scratch = warmpool.tile([PCHUNK, OUT], bf16, tag="wu")
            nc.gpsimd.memset(scratch[:], 0)
            ps_warm = pswarm.tile([PCHUNK, OUT], f32, tag="psw")
            for _ in range(10):
                nc.tensor.matmul(ps_warm[:], scratch[:, :PCHUNK], scratch[:],
                                 start=True, stop=True, skip_group_check=True)

            xh_sb = xpool.tile([PCHUNK, KC * SLOTS], bf16, tag="xh")
            nc.gpsimd.dma_start(xh_sb[:], xth[:])
            xl_sb = xpool.tile([PCHUNK, KC * SLOTS], bf16, tag="xl")
            nc.gpsimd.dma_start(xl_sb[:], xtl[:])

            ysbs = []
            for g in range(GROUPS):
                ysb_g = ypool.tile([128, OUT], f32, tag="y")
                ysbs.append(ysb_g)

            for j in range(BPC):
                wh_sb = wpool.tile([PCHUNK, KC * OUT], bf16, tag="w")
                nc.sync.dma_start(wh_sb[:], wh[j])
                wl_sb = wpool.tile([PCHUNK, KC * OUT], bf16, tag="w")
                nc.sync.dma_start(wl_sb[:], wl[j])

                ps = pspool.tile([CAP, OUT], f32, tag="ps")
                nmm = 3 * KC
                i = 0
                for kc in range(KC):
                    xs = slice(kc * SLOTS + j * CAP, kc * SLOTS + (j + 1) * CAP)
                    ws = slice(kc * OUT, (kc + 1) * OUT)
                    for a_sb, b_sb in ((xh_sb, wh_sb), (xh_sb, wl_sb),
                                       (xl_sb, wh_sb)):
                        nc.tensor.matmul(ps[:], a_sb[:, xs], b_sb[:, ws],
                                         start=(i == 0), stop=(i == nmm - 1))
                        i += 1
                g, q = divmod(j, 128 // CAP)
                nc.vector.tensor_copy(
                    ysbs[g][q * CAP:(q + 1) * CAP, :], ps[:])

            for g in range(GROUPS):
                nc.sync.dma_start(y[g * 128:(g + 1) * 128, :], ysbs[g][:])
    nc.compile()
    return nc


def _get_nc():
    if "nc" not in _cache:
        _cache["nc"] = _build_nc()
    return _cache["nc"]


def _split_hilo(a32):
    """fp32 array -> (hi, lo) bf16 halves with a32 ~= hi + lo."""
    import ml_dtypes
    bf = ml_dtypes.bfloat16
    hi = a32.astype(bf)
    lo = (a32 - hi.astype(np.float32)).astype(bf)
    return hi, lo


def _swizzle_x(xt):
    """[IN, SLOTS] -> [128, KC*SLOTS] with free index (kc, slot)."""
    return np.ascontiguousarray(
        xt.reshape(KC, PCHUNK, SLOTS).transpose(1, 0, 2).reshape(
            PCHUNK, KC * SLOTS))


def _swizzle_w(w):
    """[BPC, IN, OUT] -> [BPC, 128, KC*OUT] with free index (kc, out)."""
    return np.ascontiguousarray(
        w.reshape(BPC, KC, PCHUNK, OUT).transpose(0, 2, 1, 3).reshape(
            BPC, PCHUNK, KC * OUT))


def _route(X, sel, prob):
    """Group token-bank pairs by bank, build per-core dispatch arrays.

    Returns (in_maps, slot_tok [NCORES,SLOTS] int64 (-1=pad), overflow list
    of (token, bank, prob))."""
    NT = X.shape[0]
    pair_tok = np.repeat(np.arange(NT, dtype=np.int64), KSEL)
    pair_bank = sel.reshape(-1)
    pair_p = prob.reshape(-1)

    order = np.argsort(pair_bank, kind="stable")
    counts = np.bincount(pair_bank, minlength=NB)
    starts = np.concatenate(([0], np.cumsum(counts)))

    slot_tok = np.full((NCORES, SLOTS), -1, dtype=np.int64)
    slot_p = np.zeros((NCORES, SLOTS), dtype=np.float32)
    overflow = []
    for b in range(NB):
        c, j = divmod(b, BPC)
        s0, s1 = starts[b], starts[b + 1]
        take = min(s1 - s0, CAP)
        idx = order[s0:s0 + take]
        slot_tok[c, j * CAP: j * CAP + take] = pair_tok[idx]
        slot_p[c, j * CAP: j * CAP + take] = pair_p[idx]
        for i in order[s0 + take:s1]:
            overflow.append((int(pair_tok[i]), b, float(pair_p[i])))
    return slot_tok, slot_p, overflow


def _combine(ys, slot_tok, X, sel, prob, weights, bias, overflow):
    NT = X.shape[0]
    out = np.zeros((NT, OUT), dtype=np.float32)
    for c in range(NCORES):
        tok = slot_tok[c]
        valid = tok >= 0
        np.add.at(out, tok[valid], ys[c][valid])
    # bias term for every pair (device computes x @ W only)
    for k in range(KSEL):
        out += prob[:, k, None] * bias[sel[:, k]]
    # exact host fallback for capacity-overflow pairs (expected: none)
    for t, b, p in overflow:
        out[t] += p * (X[t] @ weights[b])
    return out


def _run_device(in_maps, trace=False, **kwargs):
    from concourse.bass_utils import run_bass_kernel_spmd
    return run_bass_kernel_spmd(_get_nc(), in_maps,
                                core_ids=list(range(NCORES)),
                                trace=trace, **kwargs)


def kernel(_trace=False, _bass_results=None, **inputs):
    tensor = np.asarray(inputs["tensor"], dtype=np.float32)
    sel = np.asarray(inputs["bank_selections"]).astype(np.int64)
    prob = np.asarray(inputs["bank_probabilities"], dtype=np.float32)
    weights = np.asarray(inputs["weights"], dtype=np.float32)
    bias = np.asarray(inputs["bias"], dtype=np.float32)

    NT = tensor.shape[0] * tensor.shape[1]
    X = tensor.reshape(NT, IN)
    sel2 = sel.reshape(NT, KSEL)
    prob2 = prob.reshape(NT, KSEL)

    slot_tok, slot_p, overflow = _route(X, sel2, prob2)

    in_maps = []
    for c in range(NCORES):
        tok = slot_tok[c]
        rows = X[np.where(tok >= 0, tok, 0)] * slot_p[c][:, None]
        xt = np.ascontiguousarray(rows.T)              # [IN, SLOTS] fp32
        xh, xl = _split_hilo(xt)
        w32 = weights[c * BPC:(c + 1) * BPC]           # (8, 512, 512) fp32
        wwh, wwl = _split_hilo(w32)
        in_maps.append({
            "xth": _swizzle_x(xh), "xtl": _swizzle_x(xl),
            "wh": _swizzle_w(wwh), "wl": _swizzle_w(wwl),
        })

    res = _run_device(in_maps, trace=_trace)
    if _bass_results is not None:
        _bass_results.append(res)
    ys = [res.results[c]["y"] for c in range(NCORES)]

    out = _combine(ys, slot_tok, X, sel2, prob2, weights, bias, overflow)
    return out.reshape(tensor.shape[0], tensor.shape[1], OUT)
